# revision 45
# baseline (speedup 1.0000x reference)
"""Complex-valued attention (nn_Attention_1) on 8 Trainium2 NeuronCores.

Math (per batch b):
  q = X @ Wq_cat, k = Y @ Wk_cat, v = Y @ Wv_cat  with X=[Q_r|Q_i], Y=[KV_r|KV_i]
  scores = qr kr^T + qi ki^T  ==  sum_x X_x (Wq_cat Wk_cat^T) Y_x^T
  probs = softmax(scores + kmask_bias); ctx = probs @ v * Q_mask
Sharding: data-parallel over B=16 -> 2 batches per core, no cross-core comm.

Precision scheme: single-pass fp16 everywhere. fp16 has an 11-bit mantissa
(same as tf32), so quantizing X/Y/M/Z to fp16 gives score error ~0.04 abs
(scores have std ~90); softmax prob shift is bounded by the score-difference
error, so only near-tie rows move, giving output rel-err ~4e-3 << the 2e-2
gate (validated against the reference in fp64/np simulation).
All matmuls accumulate in fp32 PSUM.  P is kept as the raw exp (rowmax
subtracted); 1/sumexp * Q_mask is one per-row scalar folded into the ctx
PSUM->SBUF copies, keeping the softmax->AV critical chain short.
Scheduling notes: the rank-1 K_mask bias is folded into the score
accumulation right after chunk 0 (and skipped entirely for all-ones masks);
P^T transposes are grouped per q-block so AV(i) depends only on softmax(i);
dummy matmuls on a zeroed tile keep the PE HAM clock warm through the
startup DMA wait and each softmax stats chain.
"""
import sys
sys.path.insert(0, '/opt/trn_rl_repo')
import numpy as np
import ml_dtypes
from contextlib import ExitStack

import concourse.bass as bass
from concourse import bacc
import concourse.mybir as mybir
import concourse.tile as tile
from concourse.bass_utils import run_bass_kernel_spmd

B, S, E = 16, 512, 32
NCORES = 8
BPC = B // NCORES           # batches per core
NCH = 16                    # 128-row chunks of the 2048-wide (x, e-cat) axis
SQT = S // 128              # 4 sq tiles per batch
CW = NCH * 512

f32 = mybir.dt.float32
f16 = mybir.dt.float16
bf16 = mybir.dt.bfloat16

LAST_EXEC_NS = None
_NC_CACHE = None


def build_nc(with_kbias=True):
    nc = bacc.Bacc()
    x16 = nc.dram_tensor("x16", [BPC, 128, CW], f16, kind="ExternalInput")
    y16 = nc.dram_tensor("y16", [BPC, 128, CW], f16, kind="ExternalInput")
    m16 = nc.dram_tensor("m16", [128, 128], f16, kind="ExternalInput")
    wv16 = nc.dram_tensor("wv16", [128, 128], f16, kind="ExternalInput")
    ident = nc.dram_tensor("ident", [128, 128], f16, kind="ExternalInput")
    kb = nc.dram_tensor("kb", [1, BPC * 512], bf16, kind="ExternalInput")
    qm = nc.dram_tensor("qm", [128, BPC * SQT], f32, kind="ExternalInput")
    out = nc.dram_tensor("out", [BPC, SQT, 128, 2048], f16, kind="ExternalOutput")

    Exp = mybir.ActivationFunctionType.Exp
    Copy = mybir.ActivationFunctionType.Copy

    with tile.TileContext(nc) as tc, ExitStack() as ctx:
        singles = ctx.enter_context(tc.tile_pool(name="singles", bufs=1))
        xpool = ctx.enter_context(tc.tile_pool(name="xpool", bufs=2))
        ypool = ctx.enter_context(tc.tile_pool(name="ypool", bufs=2))
        zpool = ctx.enter_context(tc.tile_pool(name="zpool", bufs=3))
        vpool = ctx.enter_context(tc.tile_pool(name="vpool", bufs=2))
        ppool = ctx.enter_context(tc.tile_pool(name="ppool", bufs=5))
        ptpool = ctx.enter_context(tc.tile_pool(name="ptpool", bufs=5))
        cpool = ctx.enter_context(tc.tile_pool(name="cpool", bufs=3))
        stats = ctx.enter_context(tc.tile_pool(name="stats", bufs=12))
        ps = ctx.enter_context(tc.tile_pool(name="ps", bufs=4, space="PSUM"))

        # critical weights first on the fast sync ring; the rest on scalar
        m_sb = singles.tile([128, 128], f16)
        nc.sync.dma_start(out=m_sb, in_=m16[:, :])
        wv_sb = singles.tile([128, 128], f16)
        nc.sync.dma_start(out=wv_sb, in_=wv16[:, :])
        ident_sb = singles.tile([128, 128], f16)
        nc.scalar.dma_start(out=ident_sb, in_=ident[:, :])
        kb_sb = singles.tile([1, BPC * 512], bf16)
        nc.scalar.dma_start(out=kb_sb, in_=kb[:, :])
        qm_sb = singles.tile([128, BPC * SQT], f32)
        nc.scalar.dma_start(out=qm_sb, in_=qm[:, :])
        ones_sb = singles.tile([1, 128], bf16)
        nc.vector.memset(ones_sb, 1.0)

        # HAM warmup: dummy matmuls on a zeroed tile while the first input
        # DMAs land, so real matmuls start at the full 2.4 GHz clock.
        warm_sb = singles.tile([128, 512], f16)
        nc.vector.memset(warm_sb, 0.0)
        warm_ps = ps.tile([128, 512], f32, tag="w", bufs=4)
        for _ in range(10):
            nc.tensor.matmul(warm_ps, warm_sb[:, 0:128], warm_sb,
                             start=True, stop=True)

        for b in range(BPC):
            x_sb = xpool.tile([128, CW], f16, tag="x")
            y_sb = ypool.tile([128, CW], f16, tag="y")
            # x on the sync HW-DGE ring, y on the gpsimd SW-DGE ring (they
            # run in parallel); the first group is split into 512-col pieces
            # so chunk j never waits behind a big transfer.
            pieces = [(c, c + 512) for c in range(0, 8192, 512)]
            for c0, c1 in pieces:
                nc.sync.dma_start(out=x_sb[:, c0:c1], in_=x16[b, :, c0:c1])
            for c0, c1 in pieces:
                nc.gpsimd.dma_start(out=y_sb[:, c0:c1], in_=y16[b, :, c0:c1])
            v_sb = vpool.tile([128, CW], f16, tag="v")
            # v natural viewed as [128, k(4), 2048]: col k*2048 + d
            v_3d = v_sb.rearrange("p (k d) -> p k d", k=4)

            psS = []
            for i in range(SQT):
                s_tile = ps.tile([128, 512], f32, tag="s", bufs=4)
                psS.append(s_tile)

            # software-pipelined chunk loop:
            #   stage A(j): Z-proj (1 matmul), psum -> fp16 copy on ACT
            #   stage B(j-1): scores (4 matmuls) + v-proj chunk
            zs = {}
            for j in range(NCH + 1):
                if j < NCH:
                    psz = ps.tile([128, 512], f32, tag="w", bufs=4)
                    nc.tensor.matmul(psz, m_sb, x_sb[:, j * 512:(j + 1) * 512],
                                     start=True, stop=True)
                    z = zpool.tile([128, 512], f16, tag="z")
                    nc.scalar.copy(z, psz)
                    zs[j] = z
                    if b == 0 and j == 0:
                        # fill the z0-copy latency at kernel start
                        for _ in range(3):
                            nc.tensor.matmul(warm_ps, warm_sb[:, 0:128],
                                             warm_sb, start=True, stop=True)

                jj = j - 1
                if jj < 0:
                    continue
                # v-proj first: its DVE copy then overlaps this chunk's score
                # matmuls, and the last one clears DVE before softmax stats.
                psv = ps.tile([128, 512], f32, tag="w", bufs=4)
                for k in range(4):
                    nc.tensor.matmul(
                        psv[:, k * 128:(k + 1) * 128],
                        y_sb[:, jj * 512 + k * 128:jj * 512 + (k + 1) * 128],
                        wv_sb, start=True, stop=True)
                nc.vector.tensor_copy(v_3d[:, :, jj * 128:(jj + 1) * 128],
                                      psv.rearrange("p (k c) -> p k c", k=4))
                z = zs.pop(jj)
                yj = y_sb[:, jj * 512:(jj + 1) * 512]
                for i in range(SQT):
                    nc.tensor.matmul(psS[i], z[:, i * 128:(i + 1) * 128], yj,
                                     start=(jj == 0), stop=(jj == NCH - 1))
                if jj == 0 and with_kbias:
                    # kbias rank-1 folded in early, off the softmax tail
                    # (skipped when K_mask is all-ones: bias identically 0)
                    for i in range(SQT):
                        nc.tensor.matmul(psS[i], ones_sb,
                                         kb_sb[:, b * 512:(b + 1) * 512],
                                         start=False, stop=False)

            # ---- softmax stats first: rowmax for every tile on DVE so the
            # exp of tile i is gated only by its own (rm, nm). ----
            negmxs = []
            for i in range(SQT):
                negmx = stats.tile([128, 1], f32, tag="negmx")
                nc.vector.reduce_max(out=negmx, in_=psS[i],
                                     axis=mybir.AxisListType.X, negate=True)
                negmxs.append(negmx)

            # keep the PE (and its HAM clock) busy during the serial
            # rowmax->exp chain: the first transposes wait on exp(0)
            warm2 = ps.tile([128, 512], f32, tag="w", bufs=4, name="warm2")
            for _ in range(7):
                nc.tensor.matmul(warm2, warm_sb[:, 0:128], warm_sb,
                                 start=True, stop=True)

            # Per-tile: exp (ACT) -> P^T transposes (PE) -> pt piece copies
            # (ACT, so DVE's rm queue never blocks them) -> 1/sumexp*qmask
            # scale (DVE, consumed by the ctx copies).  P is raw exp;
            # normalization is applied at the ctx PSUM->SBUF copy.
            scale_tiles = {}
            pt_tiles = {}

            def emit_softmax_T(i):
                # exp in two column-halves: the first transposes can start
                # after only half the exp latency.
                p_i = ppool.tile([128, 512], f16, tag="p", name="p_i")
                sum_a = stats.tile([128, 1], f32, tag="sumexp", name="sum_a")
                sum_b = stats.tile([128, 1], f32, tag="sumexp", name="sum_b")
                nc.scalar.activation(p_i[:, 0:256], psS[i][:, 0:256], Exp,
                                     bias=negmxs[i], scale=1.0, accum_out=sum_a)
                nc.scalar.activation(p_i[:, 256:512], psS[i][:, 256:512], Exp,
                                     bias=negmxs[i], scale=1.0, accum_out=sum_b)
                pt_i = ptpool.tile([128, 512], f16, tag="pt", name="pt_i")
                pspt = ps.tile([128, 512], f16, tag="w", bufs=4, name="pspt")
                for k in range(SQT):
                    nc.tensor.transpose(
                        pspt[:, k * 128:(k + 1) * 128],
                        p_i[:, k * 128:(k + 1) * 128],
                        ident_sb)
                for k in range(SQT):
                    nc.scalar.copy(pt_i[:, k * 128:(k + 1) * 128],
                                   pspt[:, k * 128:(k + 1) * 128])
                sumexp = stats.tile([128, 1], f32, tag="sumt", name="sumexp")
                nc.vector.tensor_add(sumexp, sum_a, sum_b)
                rsum = stats.tile([128, 1], f32, tag="rsum", name="rsum")
                nc.vector.reciprocal(rsum, sumexp)
                scale_i = stats.tile([128, 1], f32, tag="scale", name="scale_i")
                nc.vector.tensor_mul(scale_i, rsum,
                                     qm_sb[:, b * SQT + i: b * SQT + i + 1])
                pt_tiles[i] = pt_i
                scale_tiles[i] = scale_i

            emit_softmax_T(0)
            for i in range(SQT):
                ctx_sb = cpool.tile([128, 2048], f16, tag="c")
                pt_i = pt_tiles.pop(i)
                for n in range(4):
                    psc = ps.tile([128, 512], f32,
                                  tag=("s" if n % 2 == 0 else "w"), bufs=4)
                    for k in range(SQT):
                        nc.tensor.matmul(
                            psc,
                            pt_i[:, k * 128:(k + 1) * 128],
                            v_3d[:, k, n * 512:(n + 1) * 512],
                            start=(k == 0), stop=(k == SQT - 1))
                    if i == 0 and n == 0 and SQT > 1:
                        emit_softmax_T(1)
                    if i == 0 and n == 2 and SQT > 2:
                        emit_softmax_T(2)
                    if i == 1 and n == 0 and SQT > 3:
                        emit_softmax_T(3)
                    last_grp = (b == BPC - 1 and i == SQT - 1)
                    if last_grp:
                        # tail: split the copy across both engines
                        nc.scalar.activation(
                            ctx_sb[:, n * 512:n * 512 + 256],
                            psc[:, 0:256], Copy, bias=0.0,
                            scale=scale_tiles[i])
                        nc.vector.tensor_scalar_mul(
                            ctx_sb[:, n * 512 + 256:(n + 1) * 512],
                            psc[:, 256:512], scale_tiles[i])
                    elif n % 2 == 0:
                        nc.scalar.activation(ctx_sb[:, n * 512:(n + 1) * 512],
                                             psc, Copy, bias=0.0,
                                             scale=scale_tiles[i])
                    else:
                        nc.vector.tensor_scalar_mul(
                            ctx_sb[:, n * 512:(n + 1) * 512], psc,
                            scale_tiles[i])
                    if last_grp:
                        # final group: half-width DMAs on the two idle HW
                        # rings so the last transfer starts right after its
                        # own half-copy
                        nc.sync.dma_start(
                            out=out[b, i, :, n * 512:n * 512 + 256],
                            in_=ctx_sb[:, n * 512:n * 512 + 256])
                        nc.scalar.dma_start(
                            out=out[b, i, :, n * 512 + 256:(n + 1) * 512],
                            in_=ctx_sb[:, n * 512 + 256:(n + 1) * 512])
                    else:
                        nc.gpsimd.dma_start(
                            out=out[b, i, :, n * 512:(n + 1) * 512],
                            in_=ctx_sb[:, n * 512:(n + 1) * 512])

    nc.compile()
    return nc


def _cat_w(wr, wi):
    """[[Wr, Wi], [-Wi, Wr]] : (e_cat 64) x (f_cat 64)."""
    top = np.concatenate([wr, wi], axis=1)
    bot = np.concatenate([-wi, wr], axis=1)
    return np.concatenate([top, bot], axis=0)


def _bd(w):
    z = np.zeros_like(w)
    return np.block([[w, z], [z, w]]).astype(np.float32)


def _prep(inputs):
    """Pure layout transforms + O(weight) algebra on host."""
    Qr, Qi = np.asarray(inputs['Q_r']), np.asarray(inputs['Q_i'])
    KVr, KVi = np.asarray(inputs['KV_r']), np.asarray(inputs['KV_i'])
    Km, Qm = np.asarray(inputs['K_mask']), np.asarray(inputs['Q_mask'])

    X = np.concatenate([Qr, Qi], axis=-1)     # [B, S, 32, 64]
    Y = np.concatenate([KVr, KVi], axis=-1)
    # X^T layout: [B, 128, 16*512] with partition p of chunk j = row j*128+p
    # of the flattened (x*64 + c) axis.
    def to_xt(A):
        At = A.transpose(0, 2, 3, 1).reshape(B, 2048, S)        # [B, (x c), S]
        At = At.reshape(B, NCH, 128, S).transpose(0, 2, 1, 3)   # [B, 128, 16, S]
        return np.ascontiguousarray(
            At.reshape(B, 128, NCH * S), np.float32).astype(np.float16)

    x16 = to_xt(X)
    y16 = to_xt(Y)

    Wq = _cat_w(np.asarray(inputs['Wq_r']), np.asarray(inputs['Wq_i']))
    Wk = _cat_w(np.asarray(inputs['Wk_r']), np.asarray(inputs['Wk_i']))
    Wv = _cat_w(np.asarray(inputs['Wv_r']), np.asarray(inputs['Wv_i']))
    M2 = (Wq.astype(np.float64) @ Wk.astype(np.float64).T).astype(np.float32)
    m16 = _bd(M2).astype(np.float16)
    wv16 = _bd(Wv.astype(np.float32)).astype(np.float16)
    ident = np.eye(128, dtype=np.float16)

    kbias = ((1.0 - Km) * -100000.0).astype(ml_dtypes.bfloat16)  # [B, S]
    in_maps = []
    for c in range(NCORES):
        bs = slice(c * BPC, (c + 1) * BPC)
        qm_c = np.ascontiguousarray(
            Qm[bs].reshape(BPC, SQT, 128).transpose(2, 0, 1)
            .reshape(128, BPC * SQT), np.float32)
        in_maps.append({
            "x16": x16[bs], "y16": y16[bs],
            "m16": m16, "wv16": wv16, "ident": ident,
            "kb": np.ascontiguousarray(kbias[bs].reshape(1, BPC * 512)),
            "qm": qm_c,
        })
    return in_maps


def kernel(_trace=False, _tmpdir=None, **inputs):
    global LAST_EXEC_NS, _NC_CACHE
    in_maps = _prep(inputs)
    # When K_mask is all ones (the spec's fill) the additive score bias is
    # identically zero, so compile the variant without the rank-1 bias
    # matmuls; any other mask uses the general variant.
    with_kbias = not np.all(np.asarray(inputs['K_mask']) == 1.0)
    if _NC_CACHE is None or _NC_CACHE[0] != with_kbias:
        _NC_CACHE = (with_kbias, build_nc(with_kbias=with_kbias))
    res = run_bass_kernel_spmd(_NC_CACHE[1], in_maps,
                               core_ids=list(range(NCORES)),
                               trace=_trace, tmpdir=_tmpdir)
    LAST_EXEC_NS = res.exec_time_ns
    outs = [np.asarray(res.results[c]["out"]) for c in range(NCORES)]
    ctx = np.concatenate(outs, axis=0).astype(np.float32)  # [B, 4, 128, 2048]
    ctx = ctx.reshape(B, S, 32, 2, 32)          # [B, S, x, (r|i), f]
    return (ctx[..., 0, :] + 1j * ctx[..., 1, :]).astype(np.complex64)


# revision 46
# speedup vs baseline: 1.0050x; 1.0050x over previous
"""Complex-valued attention (nn_Attention_1) on 8 Trainium2 NeuronCores.

Math (per batch b):
  q = X @ Wq_cat, k = Y @ Wk_cat, v = Y @ Wv_cat  with X=[Q_r|Q_i], Y=[KV_r|KV_i]
  scores = qr kr^T + qi ki^T  ==  sum_x X_x (Wq_cat Wk_cat^T) Y_x^T
  probs = softmax(scores + kmask_bias); ctx = probs @ v * Q_mask
Sharding: data-parallel over B=16 -> 2 batches per core, no cross-core comm.

Precision scheme: single-pass fp16 everywhere. fp16 has an 11-bit mantissa
(same as tf32), so quantizing X/Y/M/Z to fp16 gives score error ~0.04 abs
(scores have std ~90); softmax prob shift is bounded by the score-difference
error, so only near-tie rows move, giving output rel-err ~4e-3 << the 2e-2
gate (validated against the reference in fp64/np simulation).
All matmuls accumulate in fp32 PSUM.  P is kept as the raw exp (rowmax
subtracted); 1/sumexp * Q_mask is one per-row scalar folded into the ctx
PSUM->SBUF copies, keeping the softmax->AV critical chain short.
Scheduling notes: the rank-1 K_mask bias is folded into the score
accumulation right after chunk 0 (and skipped entirely for all-ones masks);
P^T transposes are grouped per q-block so AV(i) depends only on softmax(i);
dummy matmuls on a zeroed tile keep the PE HAM clock warm through the
startup DMA wait and each softmax stats chain.
"""
import sys
sys.path.insert(0, '/opt/trn_rl_repo')
import numpy as np
import ml_dtypes
from contextlib import ExitStack

import concourse.bass as bass
from concourse import bacc
import concourse.mybir as mybir
import concourse.tile as tile
from concourse.bass_utils import run_bass_kernel_spmd

B, S, E = 16, 512, 32
NCORES = 8
BPC = B // NCORES           # batches per core
NCH = 16                    # 128-row chunks of the 2048-wide (x, e-cat) axis
SQT = S // 128              # 4 sq tiles per batch
CW = NCH * 512

f32 = mybir.dt.float32
f16 = mybir.dt.float16
bf16 = mybir.dt.bfloat16

LAST_EXEC_NS = None
_NC_CACHE = None


def build_nc(with_kbias=True):
    nc = bacc.Bacc()
    x16 = nc.dram_tensor("x16", [BPC, 128, CW], f16, kind="ExternalInput")
    y16 = nc.dram_tensor("y16", [BPC, 128, CW], f16, kind="ExternalInput")
    m16 = nc.dram_tensor("m16", [128, 128], f16, kind="ExternalInput")
    wv16 = nc.dram_tensor("wv16", [128, 128], f16, kind="ExternalInput")
    ident = nc.dram_tensor("ident", [128, 128], f16, kind="ExternalInput")
    kb = nc.dram_tensor("kb", [1, BPC * 512], bf16, kind="ExternalInput")
    qm = nc.dram_tensor("qm", [128, BPC * SQT], f32, kind="ExternalInput")
    out = nc.dram_tensor("out", [BPC, SQT, 128, 2048], f16, kind="ExternalOutput")

    Exp = mybir.ActivationFunctionType.Exp
    Copy = mybir.ActivationFunctionType.Copy

    with tile.TileContext(nc) as tc, ExitStack() as ctx:
        singles = ctx.enter_context(tc.tile_pool(name="singles", bufs=1))
        xpool = ctx.enter_context(tc.tile_pool(name="xpool", bufs=2))
        ypool = ctx.enter_context(tc.tile_pool(name="ypool", bufs=2))
        zpool = ctx.enter_context(tc.tile_pool(name="zpool", bufs=3))
        vpool = ctx.enter_context(tc.tile_pool(name="vpool", bufs=2))
        ppool = ctx.enter_context(tc.tile_pool(name="ppool", bufs=5))
        ptpool = ctx.enter_context(tc.tile_pool(name="ptpool", bufs=5))
        cpool = ctx.enter_context(tc.tile_pool(name="cpool", bufs=3))
        stats = ctx.enter_context(tc.tile_pool(name="stats", bufs=12))
        ps = ctx.enter_context(tc.tile_pool(name="ps", bufs=4, space="PSUM"))

        # critical weights first on the fast sync ring; the rest on scalar
        m_sb = singles.tile([128, 128], f16)
        nc.sync.dma_start(out=m_sb, in_=m16[:, :])
        wv_sb = singles.tile([128, 128], f16)
        nc.sync.dma_start(out=wv_sb, in_=wv16[:, :])
        ident_sb = singles.tile([128, 128], f16)
        nc.scalar.dma_start(out=ident_sb, in_=ident[:, :])
        kb_sb = singles.tile([1, BPC * 512], bf16)
        nc.scalar.dma_start(out=kb_sb, in_=kb[:, :])
        qm_sb = singles.tile([128, BPC * SQT], f32)
        nc.scalar.dma_start(out=qm_sb, in_=qm[:, :])
        ones_sb = singles.tile([1, 128], bf16)
        nc.vector.memset(ones_sb, 1.0)

        # HAM warmup: dummy matmuls on a zeroed tile while the first input
        # DMAs land, so real matmuls start at the full 2.4 GHz clock.
        warm_sb = singles.tile([128, 512], f16)
        nc.vector.memset(warm_sb, 0.0)
        warm_ps = ps.tile([128, 512], f32, tag="w", bufs=4)
        for _ in range(10):
            nc.tensor.matmul(warm_ps, warm_sb[:, 0:128], warm_sb,
                             start=True, stop=True)

        for b in range(BPC):
            x_sb = xpool.tile([128, CW], f16, tag="x")
            y_sb = ypool.tile([128, CW], f16, tag="y")
            # x on the sync HW-DGE ring, y on the gpsimd SW-DGE ring (they
            # run in parallel); the first group is split into 512-col pieces
            # so chunk j never waits behind a big transfer.
            pieces = [(c, c + 512) for c in range(0, 8192, 512)]
            for c0, c1 in pieces:
                nc.sync.dma_start(out=x_sb[:, c0:c1], in_=x16[b, :, c0:c1])
            for c0, c1 in pieces:
                nc.gpsimd.dma_start(out=y_sb[:, c0:c1], in_=y16[b, :, c0:c1])
            v_sb = vpool.tile([128, CW], f16, tag="v")
            # v natural viewed as [128, k(4), 2048]: col k*2048 + d
            v_3d = v_sb.rearrange("p (k d) -> p k d", k=4)

            psS = []
            for i in range(SQT):
                s_tile = ps.tile([128, 512], f32, tag="s", bufs=4)
                psS.append(s_tile)

            # software-pipelined chunk loop:
            #   stage A(j): Z-proj (1 matmul), psum -> fp16 copy on ACT
            #   stage B(j-1): scores (4 matmuls) + v-proj chunk
            zs = {}
            for j in range(NCH + 1):
                if j < NCH:
                    psz = ps.tile([128, 512], f32, tag="w", bufs=4)
                    nc.tensor.matmul(psz, m_sb, x_sb[:, j * 512:(j + 1) * 512],
                                     start=True, stop=True)
                    z = zpool.tile([128, 512], f16, tag="z")
                    nc.scalar.copy(z, psz)
                    zs[j] = z
                    if b == 0 and j == 0:
                        # fill the z0-copy latency at kernel start
                        for _ in range(3):
                            nc.tensor.matmul(warm_ps, warm_sb[:, 0:128],
                                             warm_sb, start=True, stop=True)

                jj = j - 1
                if jj < 0:
                    continue
                # v-proj first: its DVE copy then overlaps this chunk's score
                # matmuls, and the last one clears DVE before softmax stats.
                psv = ps.tile([128, 512], f32, tag="w", bufs=4)
                for k in range(4):
                    nc.tensor.matmul(
                        psv[:, k * 128:(k + 1) * 128],
                        y_sb[:, jj * 512 + k * 128:jj * 512 + (k + 1) * 128],
                        wv_sb, start=True, stop=True)
                nc.vector.tensor_copy(v_3d[:, :, jj * 128:(jj + 1) * 128],
                                      psv.rearrange("p (k c) -> p k c", k=4))
                z = zs.pop(jj)
                yj = y_sb[:, jj * 512:(jj + 1) * 512]
                for i in range(SQT):
                    nc.tensor.matmul(psS[i], z[:, i * 128:(i + 1) * 128], yj,
                                     start=(jj == 0), stop=(jj == NCH - 1))
                if jj == 0 and with_kbias:
                    # kbias rank-1 folded in early, off the softmax tail
                    # (skipped when K_mask is all-ones: bias identically 0)
                    for i in range(SQT):
                        nc.tensor.matmul(psS[i], ones_sb,
                                         kb_sb[:, b * 512:(b + 1) * 512],
                                         start=False, stop=False)

            # ---- softmax stats first: rowmax for every tile on DVE so the
            # exp of tile i is gated only by its own (rm, nm). ----
            negmxs = []
            for i in range(SQT):
                negmx = stats.tile([128, 1], f32, tag="negmx")
                nc.vector.reduce_max(out=negmx, in_=psS[i],
                                     axis=mybir.AxisListType.X, negate=True)
                negmxs.append(negmx)

            # keep the PE (and its HAM clock) busy during the serial
            # rowmax->exp chain: the first transposes wait on exp(0)
            warm2 = ps.tile([128, 512], f32, tag="w", bufs=4, name="warm2")
            for _ in range(7):
                nc.tensor.matmul(warm2, warm_sb[:, 0:128], warm_sb,
                                 start=True, stop=True)

            # Per-tile: exp (ACT) -> P^T transposes (PE) -> pt piece copies
            # (ACT, so DVE's rm queue never blocks them) -> 1/sumexp*qmask
            # scale (DVE, consumed by the ctx copies).  P is raw exp;
            # normalization is applied at the ctx PSUM->SBUF copy.
            scale_tiles = {}
            pt_tiles = {}

            def emit_softmax_T(i):
                # exp in two column-halves: the first transposes can start
                # after only half the exp latency.
                p_i = ppool.tile([128, 512], f16, tag="p", name="p_i")
                sum_a = stats.tile([128, 1], f32, tag="sumexp", name="sum_a")
                sum_b = stats.tile([128, 1], f32, tag="sumexp", name="sum_b")
                nc.scalar.activation(p_i[:, 0:256], psS[i][:, 0:256], Exp,
                                     bias=negmxs[i], scale=1.0, accum_out=sum_a)
                nc.scalar.activation(p_i[:, 256:512], psS[i][:, 256:512], Exp,
                                     bias=negmxs[i], scale=1.0, accum_out=sum_b)
                pt_i = ptpool.tile([128, 512], f16, tag="pt", name="pt_i")
                pspt = ps.tile([128, 512], f16, tag="w", bufs=4, name="pspt")
                for k in range(SQT):
                    nc.tensor.transpose(
                        pspt[:, k * 128:(k + 1) * 128],
                        p_i[:, k * 128:(k + 1) * 128],
                        ident_sb)
                for k in range(SQT):
                    nc.scalar.copy(pt_i[:, k * 128:(k + 1) * 128],
                                   pspt[:, k * 128:(k + 1) * 128])
                sumexp = stats.tile([128, 1], f32, tag="sumt", name="sumexp")
                nc.vector.tensor_add(sumexp, sum_a, sum_b)
                rsum = stats.tile([128, 1], f32, tag="rsum", name="rsum")
                nc.vector.reciprocal(rsum, sumexp)
                scale_i = stats.tile([128, 1], f32, tag="scale", name="scale_i")
                nc.vector.tensor_mul(scale_i, rsum,
                                     qm_sb[:, b * SQT + i: b * SQT + i + 1])
                pt_tiles[i] = pt_i
                scale_tiles[i] = scale_i

            emit_softmax_T(0)
            for i in range(SQT):
                ctx_sb = cpool.tile([128, 2048], f16, tag="c")
                pt_i = pt_tiles.pop(i)
                for n in range(4):
                    psc = ps.tile([128, 512], f32,
                                  tag=("s" if n % 2 == 0 else "w"), bufs=4)
                    for k in range(SQT):
                        nc.tensor.matmul(
                            psc,
                            pt_i[:, k * 128:(k + 1) * 128],
                            v_3d[:, k, n * 512:(n + 1) * 512],
                            start=(k == 0), stop=(k == SQT - 1))
                    if i == 0 and n == 0 and SQT > 1:
                        emit_softmax_T(1)
                    if i == 0 and n == 2 and SQT > 2:
                        emit_softmax_T(2)
                    if i == 1 and n == 0 and SQT > 3:
                        emit_softmax_T(3)
                    last_grp = (b == BPC - 1 and i == SQT - 1)
                    if last_grp:
                        # tail: split the copy across both engines
                        nc.scalar.activation(
                            ctx_sb[:, n * 512:n * 512 + 256],
                            psc[:, 0:256], Copy, bias=0.0,
                            scale=scale_tiles[i])
                        nc.vector.tensor_scalar_mul(
                            ctx_sb[:, n * 512 + 256:(n + 1) * 512],
                            psc[:, 256:512], scale_tiles[i])
                    elif n % 2 == 0:
                        nc.scalar.activation(ctx_sb[:, n * 512:(n + 1) * 512],
                                             psc, Copy, bias=0.0,
                                             scale=scale_tiles[i])
                    else:
                        nc.vector.tensor_scalar_mul(
                            ctx_sb[:, n * 512:(n + 1) * 512], psc,
                            scale_tiles[i])
                    if last_grp:
                        # final group: the two idle HW rings in parallel
                        eng = nc.sync if n % 2 == 0 else nc.scalar
                        eng.dma_start(
                            out=out[b, i, :, n * 512:(n + 1) * 512],
                            in_=ctx_sb[:, n * 512:(n + 1) * 512])
                    else:
                        nc.gpsimd.dma_start(
                            out=out[b, i, :, n * 512:(n + 1) * 512],
                            in_=ctx_sb[:, n * 512:(n + 1) * 512])

    nc.compile()
    return nc


def _cat_w(wr, wi):
    """[[Wr, Wi], [-Wi, Wr]] : (e_cat 64) x (f_cat 64)."""
    top = np.concatenate([wr, wi], axis=1)
    bot = np.concatenate([-wi, wr], axis=1)
    return np.concatenate([top, bot], axis=0)


def _bd(w):
    z = np.zeros_like(w)
    return np.block([[w, z], [z, w]]).astype(np.float32)


def _prep(inputs):
    """Pure layout transforms + O(weight) algebra on host."""
    Qr, Qi = np.asarray(inputs['Q_r']), np.asarray(inputs['Q_i'])
    KVr, KVi = np.asarray(inputs['KV_r']), np.asarray(inputs['KV_i'])
    Km, Qm = np.asarray(inputs['K_mask']), np.asarray(inputs['Q_mask'])

    X = np.concatenate([Qr, Qi], axis=-1)     # [B, S, 32, 64]
    Y = np.concatenate([KVr, KVi], axis=-1)
    # X^T layout: [B, 128, 16*512] with partition p of chunk j = row j*128+p
    # of the flattened (x*64 + c) axis.
    def to_xt(A):
        At = A.transpose(0, 2, 3, 1).reshape(B, 2048, S)        # [B, (x c), S]
        At = At.reshape(B, NCH, 128, S).transpose(0, 2, 1, 3)   # [B, 128, 16, S]
        return np.ascontiguousarray(
            At.reshape(B, 128, NCH * S), np.float32).astype(np.float16)

    x16 = to_xt(X)
    y16 = to_xt(Y)

    Wq = _cat_w(np.asarray(inputs['Wq_r']), np.asarray(inputs['Wq_i']))
    Wk = _cat_w(np.asarray(inputs['Wk_r']), np.asarray(inputs['Wk_i']))
    Wv = _cat_w(np.asarray(inputs['Wv_r']), np.asarray(inputs['Wv_i']))
    M2 = (Wq.astype(np.float64) @ Wk.astype(np.float64).T).astype(np.float32)
    m16 = _bd(M2).astype(np.float16)
    wv16 = _bd(Wv.astype(np.float32)).astype(np.float16)
    ident = np.eye(128, dtype=np.float16)

    kbias = ((1.0 - Km) * -100000.0).astype(ml_dtypes.bfloat16)  # [B, S]
    in_maps = []
    for c in range(NCORES):
        bs = slice(c * BPC, (c + 1) * BPC)
        qm_c = np.ascontiguousarray(
            Qm[bs].reshape(BPC, SQT, 128).transpose(2, 0, 1)
            .reshape(128, BPC * SQT), np.float32)
        in_maps.append({
            "x16": x16[bs], "y16": y16[bs],
            "m16": m16, "wv16": wv16, "ident": ident,
            "kb": np.ascontiguousarray(kbias[bs].reshape(1, BPC * 512)),
            "qm": qm_c,
        })
    return in_maps


def kernel(_trace=False, _tmpdir=None, **inputs):
    global LAST_EXEC_NS, _NC_CACHE
    in_maps = _prep(inputs)
    # When K_mask is all ones (the spec's fill) the additive score bias is
    # identically zero, so compile the variant without the rank-1 bias
    # matmuls; any other mask uses the general variant.
    with_kbias = not np.all(np.asarray(inputs['K_mask']) == 1.0)
    if _NC_CACHE is None or _NC_CACHE[0] != with_kbias:
        _NC_CACHE = (with_kbias, build_nc(with_kbias=with_kbias))
    res = run_bass_kernel_spmd(_NC_CACHE[1], in_maps,
                               core_ids=list(range(NCORES)),
                               trace=_trace, tmpdir=_tmpdir)
    LAST_EXEC_NS = res.exec_time_ns
    outs = [np.asarray(res.results[c]["out"]) for c in range(NCORES)]
    ctx = np.concatenate(outs, axis=0).astype(np.float32)  # [B, 4, 128, 2048]
    ctx = ctx.reshape(B, S, 32, 2, 32)          # [B, S, x, (r|i), f]
    return (ctx[..., 0, :] + 1j * ctx[..., 1, :]).astype(np.complex64)


# revision 47
# speedup vs baseline: 1.0096x; 1.0046x over previous
"""Complex-valued attention (nn_Attention_1) on 8 Trainium2 NeuronCores.

Math (per batch b):
  q = X @ Wq_cat, k = Y @ Wk_cat, v = Y @ Wv_cat  with X=[Q_r|Q_i], Y=[KV_r|KV_i]
  scores = qr kr^T + qi ki^T  ==  sum_x X_x (Wq_cat Wk_cat^T) Y_x^T
  probs = softmax(scores + kmask_bias); ctx = probs @ v * Q_mask
Sharding: data-parallel over B=16 -> 2 batches per core, no cross-core comm.

Precision scheme: single-pass fp16 everywhere. fp16 has an 11-bit mantissa
(same as tf32), so quantizing X/Y/M/Z to fp16 gives score error ~0.04 abs
(scores have std ~90); softmax prob shift is bounded by the score-difference
error, so only near-tie rows move, giving output rel-err ~4e-3 << the 2e-2
gate (validated against the reference in fp64/np simulation).
All matmuls accumulate in fp32 PSUM.  P is kept as the raw exp (rowmax
subtracted); 1/sumexp * Q_mask is one per-row scalar folded into the ctx
PSUM->SBUF copies, keeping the softmax->AV critical chain short.
Scheduling notes: the rank-1 K_mask bias is folded into the score
accumulation right after chunk 0 (and skipped entirely for all-ones masks);
P^T transposes are grouped per q-block so AV(i) depends only on softmax(i);
dummy matmuls on a zeroed tile keep the PE HAM clock warm through the
startup DMA wait and each softmax stats chain.
"""
import sys
sys.path.insert(0, '/opt/trn_rl_repo')
import numpy as np
import ml_dtypes
from contextlib import ExitStack

import concourse.bass as bass
from concourse import bacc
import concourse.mybir as mybir
import concourse.tile as tile
from concourse.bass_utils import run_bass_kernel_spmd

B, S, E = 16, 512, 32
NCORES = 8
BPC = B // NCORES           # batches per core
NCH = 16                    # 128-row chunks of the 2048-wide (x, e-cat) axis
SQT = S // 128              # 4 sq tiles per batch
CW = NCH * 512

f32 = mybir.dt.float32
f16 = mybir.dt.float16
bf16 = mybir.dt.bfloat16

LAST_EXEC_NS = None
_NC_CACHE = None


def build_nc(with_kbias=True):
    nc = bacc.Bacc()
    x16 = nc.dram_tensor("x16", [BPC, 128, CW], f16, kind="ExternalInput")
    y16 = nc.dram_tensor("y16", [BPC, 128, CW], f16, kind="ExternalInput")
    m16 = nc.dram_tensor("m16", [128, 128], f16, kind="ExternalInput")
    wv16 = nc.dram_tensor("wv16", [128, 128], f16, kind="ExternalInput")
    ident = nc.dram_tensor("ident", [128, 128], f16, kind="ExternalInput")
    kb = nc.dram_tensor("kb", [1, BPC * 512], bf16, kind="ExternalInput")
    qm = nc.dram_tensor("qm", [128, BPC * SQT], f32, kind="ExternalInput")
    out = nc.dram_tensor("out", [BPC, SQT, 128, 2048], f16, kind="ExternalOutput")

    Exp = mybir.ActivationFunctionType.Exp
    Copy = mybir.ActivationFunctionType.Copy

    with tile.TileContext(nc) as tc, ExitStack() as ctx:
        singles = ctx.enter_context(tc.tile_pool(name="singles", bufs=1))
        xpool = ctx.enter_context(tc.tile_pool(name="xpool", bufs=2))
        ypool = ctx.enter_context(tc.tile_pool(name="ypool", bufs=2))
        zpool = ctx.enter_context(tc.tile_pool(name="zpool", bufs=3))
        vpool = ctx.enter_context(tc.tile_pool(name="vpool", bufs=2))
        ppool = ctx.enter_context(tc.tile_pool(name="ppool", bufs=5))
        ptpool = ctx.enter_context(tc.tile_pool(name="ptpool", bufs=5))
        cpool = ctx.enter_context(tc.tile_pool(name="cpool", bufs=3))
        stats = ctx.enter_context(tc.tile_pool(name="stats", bufs=12))
        ps = ctx.enter_context(tc.tile_pool(name="ps", bufs=4, space="PSUM"))

        # M first on the sync ring (gates Z0); everything else on scalar so
        # the x pieces queue right behind M.  wv is needed ~2us later than M
        # and lands in time from the scalar ring.
        m_sb = singles.tile([128, 128], f16)
        nc.sync.dma_start(out=m_sb, in_=m16[:, :])
        wv_sb = singles.tile([128, 128], f16)
        nc.scalar.dma_start(out=wv_sb, in_=wv16[:, :])
        qm_sb = singles.tile([128, BPC * SQT], f32)
        nc.scalar.dma_start(out=qm_sb, in_=qm[:, :])
        kb_sb = singles.tile([1, BPC * 512], bf16)
        nc.scalar.dma_start(out=kb_sb, in_=kb[:, :])
        ident_sb = singles.tile([128, 128], f16)
        nc.scalar.dma_start(out=ident_sb, in_=ident[:, :])
        ones_sb = singles.tile([1, 128], bf16)
        nc.vector.memset(ones_sb, 1.0)

        # HAM warmup: dummy matmuls on a zeroed tile while the first input
        # DMAs land, so real matmuls start at the full 2.4 GHz clock.
        warm_sb = singles.tile([128, 512], f16)
        nc.vector.memset(warm_sb, 0.0)
        warm_ps = ps.tile([128, 512], f32, tag="w", bufs=4)
        for _ in range(10):
            nc.tensor.matmul(warm_ps, warm_sb[:, 0:128], warm_sb,
                             start=True, stop=True)

        for b in range(BPC):
            x_sb = xpool.tile([128, CW], f16, tag="x")
            y_sb = ypool.tile([128, CW], f16, tag="y")
            # x on the sync HW-DGE ring, y on the gpsimd SW-DGE ring (they
            # run in parallel); the first group is split into 512-col pieces
            # so chunk j never waits behind a big transfer.
            pieces = [(c, c + 512) for c in range(0, 8192, 512)]
            for c0, c1 in pieces:
                nc.sync.dma_start(out=x_sb[:, c0:c1], in_=x16[b, :, c0:c1])
            for c0, c1 in pieces:
                nc.gpsimd.dma_start(out=y_sb[:, c0:c1], in_=y16[b, :, c0:c1])
            v_sb = vpool.tile([128, CW], f16, tag="v")
            # v natural viewed as [128, k(4), 2048]: col k*2048 + d
            v_3d = v_sb.rearrange("p (k d) -> p k d", k=4)

            psS = []
            for i in range(SQT):
                s_tile = ps.tile([128, 512], f32, tag="s", bufs=4)
                psS.append(s_tile)

            # software-pipelined chunk loop:
            #   stage A(j): Z-proj (1 matmul), psum -> fp16 copy on ACT
            #   stage B(j-1): scores (4 matmuls) + v-proj chunk
            zs = {}
            for j in range(NCH + 1):
                if j < NCH:
                    psz = ps.tile([128, 512], f32, tag="w", bufs=4)
                    nc.tensor.matmul(psz, m_sb, x_sb[:, j * 512:(j + 1) * 512],
                                     start=True, stop=True)
                    z = zpool.tile([128, 512], f16, tag="z")
                    nc.scalar.copy(z, psz)
                    zs[j] = z
                    if b == 0 and j == 0:
                        # fill the z0-copy latency at kernel start
                        for _ in range(3):
                            nc.tensor.matmul(warm_ps, warm_sb[:, 0:128],
                                             warm_sb, start=True, stop=True)

                jj = j - 1
                if jj < 0:
                    continue
                # v-proj first: its DVE copy then overlaps this chunk's score
                # matmuls, and the last one clears DVE before softmax stats.
                psv = ps.tile([128, 512], f32, tag="w", bufs=4)
                for k in range(4):
                    nc.tensor.matmul(
                        psv[:, k * 128:(k + 1) * 128],
                        y_sb[:, jj * 512 + k * 128:jj * 512 + (k + 1) * 128],
                        wv_sb, start=True, stop=True)
                nc.vector.tensor_copy(v_3d[:, :, jj * 128:(jj + 1) * 128],
                                      psv.rearrange("p (k c) -> p k c", k=4))
                z = zs.pop(jj)
                yj = y_sb[:, jj * 512:(jj + 1) * 512]
                for i in range(SQT):
                    nc.tensor.matmul(psS[i], z[:, i * 128:(i + 1) * 128], yj,
                                     start=(jj == 0), stop=(jj == NCH - 1))
                if jj == 0 and with_kbias:
                    # kbias rank-1 folded in early, off the softmax tail
                    # (skipped when K_mask is all-ones: bias identically 0)
                    for i in range(SQT):
                        nc.tensor.matmul(psS[i], ones_sb,
                                         kb_sb[:, b * 512:(b + 1) * 512],
                                         start=False, stop=False)

            # ---- softmax stats first: rowmax for every tile on DVE so the
            # exp of tile i is gated only by its own (rm, nm). ----
            negmxs = []
            for i in range(SQT):
                negmx = stats.tile([128, 1], f32, tag="negmx")
                nc.vector.reduce_max(out=negmx, in_=psS[i],
                                     axis=mybir.AxisListType.X, negate=True)
                negmxs.append(negmx)

            # keep the PE (and its HAM clock) busy during the serial
            # rowmax->exp chain: the first transposes wait on exp(0)
            warm2 = ps.tile([128, 512], f32, tag="w", bufs=4, name="warm2")
            for _ in range(7):
                nc.tensor.matmul(warm2, warm_sb[:, 0:128], warm_sb,
                                 start=True, stop=True)

            # Per-tile: exp (ACT) -> P^T transposes (PE) -> pt piece copies
            # (ACT, so DVE's rm queue never blocks them) -> 1/sumexp*qmask
            # scale (DVE, consumed by the ctx copies).  P is raw exp;
            # normalization is applied at the ctx PSUM->SBUF copy.
            scale_tiles = {}
            pt_tiles = {}

            def emit_softmax_T(i):
                # exp in two column-halves: the first transposes can start
                # after only half the exp latency.
                p_i = ppool.tile([128, 512], f16, tag="p", name="p_i")
                sum_a = stats.tile([128, 1], f32, tag="sumexp", name="sum_a")
                sum_b = stats.tile([128, 1], f32, tag="sumexp", name="sum_b")
                nc.scalar.activation(p_i[:, 0:256], psS[i][:, 0:256], Exp,
                                     bias=negmxs[i], scale=1.0, accum_out=sum_a)
                nc.scalar.activation(p_i[:, 256:512], psS[i][:, 256:512], Exp,
                                     bias=negmxs[i], scale=1.0, accum_out=sum_b)
                pt_i = ptpool.tile([128, 512], f16, tag="pt", name="pt_i")
                pspt = ps.tile([128, 512], f16, tag="w", bufs=4, name="pspt")
                for k in range(SQT):
                    nc.tensor.transpose(
                        pspt[:, k * 128:(k + 1) * 128],
                        p_i[:, k * 128:(k + 1) * 128],
                        ident_sb)
                for k in range(SQT):
                    nc.scalar.copy(pt_i[:, k * 128:(k + 1) * 128],
                                   pspt[:, k * 128:(k + 1) * 128])
                sumexp = stats.tile([128, 1], f32, tag="sumt", name="sumexp")
                nc.vector.tensor_add(sumexp, sum_a, sum_b)
                rsum = stats.tile([128, 1], f32, tag="rsum", name="rsum")
                nc.vector.reciprocal(rsum, sumexp)
                scale_i = stats.tile([128, 1], f32, tag="scale", name="scale_i")
                nc.vector.tensor_mul(scale_i, rsum,
                                     qm_sb[:, b * SQT + i: b * SQT + i + 1])
                pt_tiles[i] = pt_i
                scale_tiles[i] = scale_i

            emit_softmax_T(0)
            for i in range(SQT):
                ctx_sb = cpool.tile([128, 2048], f16, tag="c")
                pt_i = pt_tiles.pop(i)
                for n in range(4):
                    psc = ps.tile([128, 512], f32,
                                  tag=("s" if n % 2 == 0 else "w"), bufs=4)
                    for k in range(SQT):
                        nc.tensor.matmul(
                            psc,
                            pt_i[:, k * 128:(k + 1) * 128],
                            v_3d[:, k, n * 512:(n + 1) * 512],
                            start=(k == 0), stop=(k == SQT - 1))
                    if i == 0 and n == 0 and SQT > 1:
                        emit_softmax_T(1)
                    if i == 0 and n == 2 and SQT > 2:
                        emit_softmax_T(2)
                    if i == 1 and n == 0 and SQT > 3:
                        emit_softmax_T(3)
                    last_grp = (b == BPC - 1 and i == SQT - 1)
                    if last_grp:
                        # tail: split the copy across both engines
                        nc.scalar.activation(
                            ctx_sb[:, n * 512:n * 512 + 256],
                            psc[:, 0:256], Copy, bias=0.0,
                            scale=scale_tiles[i])
                        nc.vector.tensor_scalar_mul(
                            ctx_sb[:, n * 512 + 256:(n + 1) * 512],
                            psc[:, 256:512], scale_tiles[i])
                    elif n % 2 == 0:
                        nc.scalar.activation(ctx_sb[:, n * 512:(n + 1) * 512],
                                             psc, Copy, bias=0.0,
                                             scale=scale_tiles[i])
                    else:
                        nc.vector.tensor_scalar_mul(
                            ctx_sb[:, n * 512:(n + 1) * 512], psc,
                            scale_tiles[i])
                    if last_grp:
                        # final group: the two idle HW rings in parallel
                        eng = nc.sync if n % 2 == 0 else nc.scalar
                        eng.dma_start(
                            out=out[b, i, :, n * 512:(n + 1) * 512],
                            in_=ctx_sb[:, n * 512:(n + 1) * 512])
                    else:
                        nc.gpsimd.dma_start(
                            out=out[b, i, :, n * 512:(n + 1) * 512],
                            in_=ctx_sb[:, n * 512:(n + 1) * 512])

    nc.compile()
    return nc


def _cat_w(wr, wi):
    """[[Wr, Wi], [-Wi, Wr]] : (e_cat 64) x (f_cat 64)."""
    top = np.concatenate([wr, wi], axis=1)
    bot = np.concatenate([-wi, wr], axis=1)
    return np.concatenate([top, bot], axis=0)


def _bd(w):
    z = np.zeros_like(w)
    return np.block([[w, z], [z, w]]).astype(np.float32)


def _prep(inputs):
    """Pure layout transforms + O(weight) algebra on host."""
    Qr, Qi = np.asarray(inputs['Q_r']), np.asarray(inputs['Q_i'])
    KVr, KVi = np.asarray(inputs['KV_r']), np.asarray(inputs['KV_i'])
    Km, Qm = np.asarray(inputs['K_mask']), np.asarray(inputs['Q_mask'])

    X = np.concatenate([Qr, Qi], axis=-1)     # [B, S, 32, 64]
    Y = np.concatenate([KVr, KVi], axis=-1)
    # X^T layout: [B, 128, 16*512] with partition p of chunk j = row j*128+p
    # of the flattened (x*64 + c) axis.
    def to_xt(A):
        At = A.transpose(0, 2, 3, 1).reshape(B, 2048, S)        # [B, (x c), S]
        At = At.reshape(B, NCH, 128, S).transpose(0, 2, 1, 3)   # [B, 128, 16, S]
        return np.ascontiguousarray(
            At.reshape(B, 128, NCH * S), np.float32).astype(np.float16)

    x16 = to_xt(X)
    y16 = to_xt(Y)

    Wq = _cat_w(np.asarray(inputs['Wq_r']), np.asarray(inputs['Wq_i']))
    Wk = _cat_w(np.asarray(inputs['Wk_r']), np.asarray(inputs['Wk_i']))
    Wv = _cat_w(np.asarray(inputs['Wv_r']), np.asarray(inputs['Wv_i']))
    M2 = (Wq.astype(np.float64) @ Wk.astype(np.float64).T).astype(np.float32)
    m16 = _bd(M2).astype(np.float16)
    wv16 = _bd(Wv.astype(np.float32)).astype(np.float16)
    ident = np.eye(128, dtype=np.float16)

    kbias = ((1.0 - Km) * -100000.0).astype(ml_dtypes.bfloat16)  # [B, S]
    in_maps = []
    for c in range(NCORES):
        bs = slice(c * BPC, (c + 1) * BPC)
        qm_c = np.ascontiguousarray(
            Qm[bs].reshape(BPC, SQT, 128).transpose(2, 0, 1)
            .reshape(128, BPC * SQT), np.float32)
        in_maps.append({
            "x16": x16[bs], "y16": y16[bs],
            "m16": m16, "wv16": wv16, "ident": ident,
            "kb": np.ascontiguousarray(kbias[bs].reshape(1, BPC * 512)),
            "qm": qm_c,
        })
    return in_maps


def kernel(_trace=False, _tmpdir=None, **inputs):
    global LAST_EXEC_NS, _NC_CACHE
    in_maps = _prep(inputs)
    # When K_mask is all ones (the spec's fill) the additive score bias is
    # identically zero, so compile the variant without the rank-1 bias
    # matmuls; any other mask uses the general variant.
    with_kbias = not np.all(np.asarray(inputs['K_mask']) == 1.0)
    if _NC_CACHE is None or _NC_CACHE[0] != with_kbias:
        _NC_CACHE = (with_kbias, build_nc(with_kbias=with_kbias))
    res = run_bass_kernel_spmd(_NC_CACHE[1], in_maps,
                               core_ids=list(range(NCORES)),
                               trace=_trace, tmpdir=_tmpdir)
    LAST_EXEC_NS = res.exec_time_ns
    outs = [np.asarray(res.results[c]["out"]) for c in range(NCORES)]
    ctx = np.concatenate(outs, axis=0).astype(np.float32)  # [B, 4, 128, 2048]
    ctx = ctx.reshape(B, S, 32, 2, 32)          # [B, S, x, (r|i), f]
    return (ctx[..., 0, :] + 1j * ctx[..., 1, :]).astype(np.complex64)
